# revision 3
# baseline (speedup 1.0000x reference)
"""MultiHeadAttention Trainium2 kernel (8-core SPMD).

Problem: B=2, S=2048, DIM=1024, 16 heads, head_dim=64, fp32.
Sharding: core c -> (batch b = c//4, head-group g = c%4, 4 heads each).
Each core computes, for its batch and 4 heads:
    q = x Wq'^T            (Wq' = SCALE*Wq, no bias -- see bias algebra below)
    k = x Wk^T             (no bias)
    v = x Wv^T             (no bias)
    S^T[k,q] = k . q       (feature-major layout)
    P^T = exp(S^T) scaled per-k by m[k] = exp(SCALE * bq . k[k])
    outT[d,q] = sum_k V'[k,d] P^T[k,q]   with V' = diag(m) [V | 1]
    attn^T = outT[0:64] / outT[64]       (per-q softmax denominator)
    partial = attn^T . P_g^T             ([seq, 1024] output-projection partial)
Host sums the 4 per-group partials per batch and adds
bv @ proj_w.T + proj_b (V-bias and proj-bias commute through softmax/proj).

Bias algebra: softmax over k of SCALE*(q0+bq).(k0+bk) equals softmax of
(SCALE*q0).k0 + SCALE*bq.k0[k] -- the q0.bk and bq.bk terms are constant in k
and drop out. The per-k term is applied multiplicatively (m[k]) by scaling V
rows, and V's bias bv adds exactly bv to every attention output row.

Schedule (v2): the PE array is kept continuously streaming -- idle gaps drop
the DVFS p-state (observed 512-col matmuls at 630ns vs 270ns) and trigger
HAM 4/8 duty cycling.  Attention runs as 16 single-head units (pair, head,
qtile) of 8 chunk-groups each; the attn.V accumulation for group g is
emitted one group behind its exp so the PE never head-of-line blocks on the
ACT engine.  All remaining work (qkv projections for pair 1, V chunks,
bias-correction, output projection split into per-pair halves) is fed into
the per-group filler slots to cover the exp-paced windows.
"""

import numpy as np

import concourse.bass as bass
import concourse.mybir as mybir
import concourse.tile as tile
from concourse import bacc
from concourse import bass_utils

F32 = mybir.dt.float32
BF16 = mybir.dt.bfloat16

P = 128
DIM = 1024
S = 2048
NH = 16
DH = 64
SCALE = 1.0 / 8.0
DC = DIM // P           # 8 contraction chunks
NST = S // 512          # 4 seq tiles of 512
NCH = S // P            # 16 kpos chunks of 128
FPC = 256               # features per core (4 heads * 64)
NWARM = 10


def build_attention_bass():
    nc = bacc.Bacc(
        "TRN2",
        target_bir_lowering=False,
        debug=False,
        enable_asserts=False,
        num_devices=8,
    )
    xT = nc.dram_tensor("xT", [DIM, S], BF16, kind="ExternalInput").ap()
    wqT = nc.dram_tensor("wqT", [DIM, FPC], BF16, kind="ExternalInput").ap()
    wkT = nc.dram_tensor("wkT", [DIM, FPC], BF16, kind="ExternalInput").ap()
    wvT = nc.dram_tensor("wvT", [DIM, FPC], BF16, kind="ExternalInput").ap()
    bqz = nc.dram_tensor("bqz", [P, 2, 2], BF16, kind="ExternalInput").ap()
    pjT = nc.dram_tensor("pjT", [FPC, DIM], BF16, kind="ExternalInput").ap()
    out = nc.dram_tensor("out", [S, DIM], F32, kind="ExternalOutput").ap()

    with tile.TileContext(nc) as tc:
        _attention_body(tc, xT, wqT, wkT, wvT, bqz, pjT, out)
    nc.compile()
    return nc


def _attention_body(tc, xT, wqT, wkT, wvT, bqz, pjT, out):
    nc = tc.nc
    Exp = mybir.ActivationFunctionType.Exp
    Mult = mybir.AluOpType.mult
    Add = mybir.AluOpType.add

    with (
        tc.tile_pool(name="const", bufs=1) as cpool,
        tc.tile_pool(name="work", bufs=1) as wpool,
        tc.tile_pool(name="exp", bufs=4) as epool,
        tc.tile_pool(name="stage", bufs=3) as spool,
        tc.tile_pool(name="pst", bufs=2, space="PSUM") as pst,      # 2x2 banks
        tc.tile_pool(name="pavp", bufs=2, space="PSUM") as pavp,    # 2x1 bank
        tc.tile_pool(name="pfl", bufs=2, space="PSUM") as pfl,      # 2x1 bank
    ):
        # ---- input loads (order = availability priority) -----------------
        wq_sb = cpool.tile([P, DC, FPC], BF16)
        nc.sync.dma_start(wq_sb, wqT.rearrange("(dc p) f -> p dc f", p=P))
        wk_sb = cpool.tile([P, DC, FPC], BF16)
        nc.sync.dma_start(wk_sb, wkT.rearrange("(dc p) f -> p dc f", p=P))
        xt = cpool.tile([P, DC, S], BF16)
        xT_r = xT.rearrange("(dc p) s -> p dc s", p=P)
        for st in range(NST):
            sl = slice(512 * st, 512 * (st + 1))
            nc.sync.dma_start(xt[:, :, sl], xT_r[:, :, sl])
        bq_sb = cpool.tile([P, 2, 2], BF16)
        nc.sync.dma_start(bq_sb, bqz)
        wv_sb = cpool.tile([P, DC, FPC], BF16)
        nc.sync.dma_start(wv_sb, wvT.rearrange("(dc p) f -> p dc f", p=P))
        pj_sb = cpool.tile([P, 2, DIM], BF16)
        nc.sync.dma_start(pj_sb, pjT.rearrange("(c p) o -> p c o", p=P))

        q_sb = wpool.tile([P, 2, S], BF16)    # [dh-in-pair, pair, seq]
        k_sb = wpool.tile([P, 2, S], BF16)
        v_sb = wpool.tile([P, NCH, 4, DH + 1], BF16)
        m_sb = wpool.tile([P, NCH, 4], F32)   # exp(c) per (kpos, chunk, head)
        at_sb = wpool.tile([P, 2, S], BF16)   # normalized attn^T
        pp_sb = wpool.tile([P, 8, 512], F32)  # proj pair-0 partials (one qt)

        # ---- PE warm-up during the DMA lead-in ---------------------------
        warm = wpool.tile([P, 512], BF16)
        nc.vector.memset(warm, 1.0)
        wps = pavp.tile([P, 512], F32, tag="av", name="warm_ps")
        for _ in range(NWARM):
            nc.tensor.matmul(wps, lhsT=warm[:, 0:P], rhs=warm,
                             start=True, stop=True)

        # ---- helpers -----------------------------------------------------
        def cast_copy(dst, src):
            nc.vector.tensor_copy(dst, src)

        def c_and_m(p):
            # c[k] = SCALE * bq_h . k0_h[k] via block-diagonal bq operand.
            c_ps = pfl.tile([P, 512], F32, tag="fl", name=f"cps{p}")
            for ch in range(NCH):
                nc.tensor.matmul(
                    c_ps[:, 2 * ch:2 * ch + 2],
                    lhsT=k_sb[:, p, P * ch:P * (ch + 1)],
                    rhs=bq_sb[:, p, :],
                    start=True,
                    stop=True,
                )
            for h in (0, 1):
                hh = 2 * p + h
                nc.scalar.activation(
                    m_sb[:, :, hh],
                    c_ps[:, 0:2 * NCH].rearrange("p (ch h) -> p ch h", h=2)[:, :, h],
                    Exp,
                )
                # denominator column of V' is exp(c) itself
                nc.vector.tensor_copy(v_sb[:, :, hh, DH], m_sb[:, :, hh])

        def scale_v(p, ch):
            nc.vector.tensor_tensor(
                v_sb[:, ch, 2 * p:2 * p + 2, 0:DH],
                v_sb[:, ch, 2 * p:2 * p + 2, 0:DH],
                m_sb[:, ch, 2 * p:2 * p + 2, None].to_broadcast([P, 2, DH]),
                Mult,
            )

        def v_chunk(ch):
            ps = pfl.tile([P, 512], F32, tag="fl", name=f"vps{ch}")
            for dc in range(DC):
                nc.tensor.matmul(
                    ps[:, 0:FPC],
                    lhsT=xt[:, dc, P * ch:P * (ch + 1)],
                    rhs=wv_sb[:, dc, :],
                    start=(dc == 0),
                    stop=(dc == DC - 1),
                )
            nc.vector.tensor_copy(
                v_sb[:, ch, :, 0:DH],
                ps[:, 0:FPC].rearrange("p (h d) -> p h d", h=4),
            )
            scale_v(0, ch)

        def qk1_tile(wsb, st, dst):
            """pair-1 q/k projection for one seq tile (filler)."""
            ps = pfl.tile([P, 512], F32, tag="fl", name=f"qk1_{st}")
            for dc in range(DC):
                nc.tensor.matmul(
                    ps,
                    lhsT=wsb[:, dc, P:2 * P],
                    rhs=xt[:, dc, 512 * st:512 * (st + 1)],
                    start=(dc == 0),
                    stop=(dc == DC - 1),
                )
            cast_copy(dst[:, 1, 512 * st:512 * (st + 1)], ps)

        def proj0(sm, nt):
            """output projection, pair-0 half -> SBUF partial."""
            ps = pfl.tile([P, 512], F32, tag="fl", name=f"p0_{sm}_{nt}")
            nc.tensor.matmul(
                ps,
                lhsT=at_sb[:, 0, P * sm:P * (sm + 1)],
                rhs=pj_sb[:, 0, 512 * nt:512 * (nt + 1)],
                start=True,
                stop=True,
            )
            nc.vector.tensor_copy(pp_sb[:, 2 * (sm % 4) + nt, :], ps)

        def proj1(sm, nt):
            """output projection, pair-1 half + combine + store."""
            ps = pfl.tile([P, 512], F32, tag="fl", name=f"p1_{sm}_{nt}")
            nc.tensor.matmul(
                ps,
                lhsT=at_sb[:, 1, P * sm:P * (sm + 1)],
                rhs=pj_sb[:, 1, 512 * nt:512 * (nt + 1)],
                start=True,
                stop=True,
            )
            stg = spool.tile([P, 512], F32, tag="out", name=f"stg{sm}_{nt}")
            nc.vector.tensor_tensor(stg, pp_sb[:, 2 * (sm % 4) + nt, :], ps, Add)
            nc.sync.dma_start(
                out[P * sm:P * (sm + 1), 512 * nt:512 * (nt + 1)], stg
            )

        # ---- lead-in: pair-0 q/k projections chasing the x DMA -----------
        # q seq-tiles go to the two "st" PSUM slots (2 banks each), k
        # seq-tiles to the "av"/"fl" slots (1 bank each) -> all 8 banks.
        qld = [pst.tile([P, 2, 512], F32, tag="st", name=f"qld{i}")
               for i in range(2)]
        kld = []
        for st in range(NST):
            sl = slice(512 * st, 512 * (st + 1))
            for dc in range(DC):
                nc.tensor.matmul(
                    qld[st // 2][:, st % 2, :],
                    lhsT=wq_sb[:, dc, 0:P],
                    rhs=xt[:, dc, sl],
                    start=(dc == 0),
                    stop=(dc == DC - 1),
                )
            kps = (pavp if st < 2 else pfl).tile(
                [P, 512], F32, tag="av" if st < 2 else "fl", name=f"kld{st}")
            kld.append(kps)
            for dc in range(DC):
                nc.tensor.matmul(
                    kps,
                    lhsT=wk_sb[:, dc, 0:P],
                    rhs=xt[:, dc, sl],
                    start=(dc == 0),
                    stop=(dc == DC - 1),
                )
            cast_copy(k_sb[:, 0, sl], kps)
            if st % 2 == 1:
                for j in (0, 1):
                    sl2 = slice(512 * (st - 1 + j), 512 * (st + j))
                    cast_copy(q_sb[:, 0, sl2], qld[st // 2][:, j, :])

        c_and_m(0)
        for ch in range(6):
            v_chunk(ch)

        # ---- attention units --------------------------------------------
        def attention_unit(p, e, qt, fillers):
            hh = 2 * p + e
            qsl = slice(512 * qt, 512 * (qt + 1))
            pav = pavp.tile([P, 512], F32, tag="av", name=f"pav{p}{e}{qt}")

            def emit_av(g, e_t):
                for j in (0, 1):
                    ch = 2 * g + j
                    nc.tensor.matmul(
                        pav[0:DH + 1, :],
                        lhsT=v_sb[:, ch, hh, :],
                        rhs=e_t[:, j, :],
                        start=(ch == 0),
                        stop=(ch == NCH - 1),
                    )

            prev = None
            for g in range(8):
                st_t = pst.tile([P, 2, 512], F32, tag="st",
                                name=f"st{p}{e}{qt}_{g}")
                for j in (0, 1):
                    ch = 2 * g + j
                    nc.tensor.matmul(
                        st_t[:, j, :],
                        lhsT=k_sb[DH * e:DH * (e + 1), p, P * ch:P * (ch + 1)],
                        rhs=q_sb[DH * e:DH * (e + 1), p, qsl],
                        start=True,
                        stop=True,
                    )
                e_t = epool.tile([P, 2, 512], BF16, tag="e",
                                 name=f"e{p}{e}{qt}_{g}")
                nc.scalar.activation(e_t, st_t, Exp)
                if fillers:
                    fillers.pop(0)()
                if prev is not None:
                    emit_av(*prev)
                prev = (g, e_t)
            emit_av(*prev)

            # drain: normalize by the accumulated denominator row
            un = spool.tile([DH + 1, 512], F32, tag="un", name=f"un{p}{e}{qt}")
            nc.vector.tensor_copy(un, pav[0:DH + 1, :])
            rec = spool.tile([1, 512], F32, tag="rec", name=f"rc{p}{e}{qt}")
            nc.vector.reciprocal(rec, un[DH:DH + 1, :])
            rb = spool.tile([DH, 512], F32, tag="rb", name=f"rb{p}{e}{qt}")
            nc.gpsimd.partition_broadcast(rb, rec)
            nc.vector.tensor_tensor(
                at_sb[DH * e:DH * (e + 1), p, qsl],
                un[0:DH, :],
                rb,
                Mult,
            )

        def cm1_and_scale():
            c_and_m(1)
            for ch in range(NCH):
                scale_v(1, ch)

        # filler thunk lists per unit index (16 units, qt-major,
        # pair/head alternating within each qt round)
        fillers = {
            0: [lambda c=c: (v_chunk(c), v_chunk(c + 1)) for c in (6, 8, 10, 12, 14)]
               + [lambda: qk1_tile(wk_sb, 0, k_sb),
                  lambda: qk1_tile(wq_sb, 0, q_sb),
                  lambda: qk1_tile(wk_sb, 1, k_sb)],
            1: [lambda: qk1_tile(wk_sb, 2, k_sb),
                lambda: qk1_tile(wk_sb, 3, k_sb),
                cm1_and_scale,
                lambda: qk1_tile(wq_sb, 1, q_sb),
                lambda: qk1_tile(wq_sb, 2, q_sb),
                lambda: qk1_tile(wq_sb, 3, q_sb)],
        }
        # proj fillers: pair-0 halves of qt r in units 4r+2/4r+3 (after both
        # pair-0 units of round r drain); pair-1 halves in units 4r+4/4r+5.
        for r in range(NST):
            sms = range(4 * r, 4 * r + 4)
            p0 = [(lambda s=sm, n=nt: proj0(s, n)) for sm in sms for nt in (0, 1)]
            p1 = [(lambda s=sm, n=nt: proj1(s, n)) for sm in sms for nt in (0, 1)]
            fillers.setdefault(4 * r + 2, []).extend(p0[:4])
            fillers.setdefault(4 * r + 3, []).extend(p0[4:])
            if r < NST - 1:
                fillers.setdefault(4 * r + 4, []).extend(p1[:4])
                fillers.setdefault(4 * r + 5, []).extend(p1[4:])
            else:
                tail = p1

        ui = 0
        for qt in range(NST):
            for (p, e) in ((0, 0), (0, 1), (1, 0), (1, 1)):
                attention_unit(p, e, qt, fillers.get(ui, []))
                ui += 1
        for t in tail:
            t()


# ----------------------------------------------------------------------------
# host-side wrapper
# ----------------------------------------------------------------------------

_NC_CACHE = {}


def _get_nc():
    if "nc" not in _NC_CACHE:
        _NC_CACHE["nc"] = build_attention_bass()
    return _NC_CACHE["nc"]


def make_in_maps(x, qkv_w, qkv_b, proj_w):
    """Build the 8 per-core input dicts (host-side sharding)."""
    import ml_dtypes

    bf16 = ml_dtypes.bfloat16
    in_maps = []
    for c in range(8):
        b, g = divmod(c, 4)
        fsl = slice(g * FPC, (g + 1) * FPC)
        wq = (SCALE * qkv_w[0 * DIM:1 * DIM][fsl]).T     # (1024, 256)
        wk = qkv_w[1 * DIM:2 * DIM][fsl].T
        wv = qkv_w[2 * DIM:3 * DIM][fsl].T
        bq = SCALE * qkv_b[0 * DIM:1 * DIM][fsl]         # (256,)
        bqz = np.zeros((P, 2, 2), np.float32)
        for p in range(2):
            for h in range(2):
                bqz[DH * h:DH * (h + 1), p, h] = bq[(2 * p + h) * DH:(2 * p + h + 1) * DH]
        pj = proj_w[:, fsl].T                            # (256, 1024)
        in_maps.append({
            "xT": np.ascontiguousarray(x[b].T).astype(bf16),
            "wqT": np.ascontiguousarray(wq).astype(bf16),
            "wkT": np.ascontiguousarray(wk).astype(bf16),
            "wvT": np.ascontiguousarray(wv).astype(bf16),
            "bqz": bqz.astype(bf16),
            "pjT": np.ascontiguousarray(pj).astype(bf16),
        })
    return in_maps


def combine_outputs(results, qkv_b, proj_w, proj_b):
    """Sum per-group partials and add the host-folded biases."""
    bv = qkv_b[2 * DIM:3 * DIM]
    host_bias = bv @ proj_w.T + proj_b                   # (1024,)
    out = np.empty((2, S, DIM), np.float32)
    for b in range(2):
        acc = np.zeros((S, DIM), np.float32)
        for g in range(4):
            acc += results[4 * b + g]["out"]
        out[b] = acc + host_bias[None, :]
    return out


def kernel(x, qkv_w, qkv_b, proj_w, proj_b):
    x = np.asarray(x, np.float32)
    qkv_w = np.asarray(qkv_w, np.float32)
    qkv_b = np.asarray(qkv_b, np.float32)
    proj_w = np.asarray(proj_w, np.float32)
    proj_b = np.asarray(proj_b, np.float32)

    nc = _get_nc()
    in_maps = make_in_maps(x, qkv_w, qkv_b, proj_w)
    res = bass_utils.run_bass_kernel_spmd(nc, in_maps, core_ids=list(range(8)))
    return combine_outputs(res.results, qkv_b, proj_w, proj_b)
